# revision 17
# baseline (speedup 1.0000x reference)
"""Single-head attention kernel for Trainium2, SPMD over 8 NeuronCores.

Problem: x [4,4096,128], Wq/Wk/Wv [128,128] -> y [4,4096,128]
  q = x @ Wq.T ; k = x @ Wk.T ; v = x @ Wv.T
  y = softmax(q k^T / sqrt(128)) v

Sharding: 8 cores = 4 batches x 2 query-halves. Each core receives its
batch's x rotated so that its 2048 queries are rows 0..2047 (attention is
invariant to permuting the key order) -> all cores run the identical NEFF
with no dynamic offsets and no collectives.

v1 design (vs. v0 bf16 baseline at 91.4us):
  - S^T = x M x^T stays bf16 on PE (M = Wq^T Wk; K=128 contraction cannot
    use fp8 DoubleRow).
  - exp is pure Schraudolph -> fp8 e5m2 bits: most tiles on ACT via
    activation(Copy, scale, bias -> int8) at ~(FD+180)/1.2 ns, the rest
    on DVE tensor_scalar from PSUM (1x). No Exp table load at all.
  - y accumulation uses fp8 DoubleRow matmuls: K=256 per instruction
    (kt pairs), lhsT = x-pair [128,2,128] e4m3, rhs = a-pair
    [128,2,512] e5m2 -> 2x fewer PE cycles than bf16.
  - The softmax denominator l is computed on the PE as well: ones-lhsT
    [128,2,1] DoubleRow matmuls accumulate per-q column sums in a
    [1,512]-per-half PSUM tile across all 16 pairs; the DVE spine tree
    of v0 is deleted entirely (DVE was the 2nd bottleneck).
  - l spread back to partitions via 8 tiny K=1 matmuls (lhsT = 1/l row
    chunk [1,128], rhs = [1,1] ones) -> [128,8] column of 1/l.
  - x^T for chunks 2..7 comes from XBAR DMA transposes on both HWDGE
    queues (frees PE); chunks 0/1 via PE transpose (needed early).
"""

import sys

sys.path.insert(0, "/opt/trn_rl_repo")

import numpy as np

import concourse.bass as bass
import concourse.mybir as mybir
from concourse import bacc
from concourse.bass_utils import run_bass_kernel_spmd
from concourse.tile import TileContext
from concourse.masks import make_identity

P = 128
N = 4096  # context length (per batch)
NQ = 2048  # queries per core
H = 128
O = 128
KT = N // P  # 32 key tiles
NC = N // 512  # 8 row chunks of 512
QBS = 1024  # query block size
QB = NQ // QBS  # 2 query blocks
NPAIR = KT // 2  # 16 kt pairs
SCALE = 1.0 / np.sqrt(128.0)
# Schraudolph exp-as-e5m2-bits (A scaled by 1/16; cancels in y/l).
# e5m2 is required: scores span z in [-10, +9] (heavy diagonal tail) and
# e4m3's dynamic range cannot cover it without Inf/NaN. uint8 output
# saturates negative bits to 0 (flush-to-zero for z < -7.6).
E5S0 = float(4.0 * np.log2(np.e) * SCALE)
E5S1 = float(4 * (15 - 4) - 0.25)

F32 = mybir.dt.float32
BF16 = mybir.dt.bfloat16
FP8E4 = mybir.dt.float8e4
FP8E5 = mybir.dt.float8e5
I16 = mybir.dt.int16
I8 = mybir.dt.int8
UI8 = mybir.dt.uint8

_cached_nc = None


def build_kernel():
    nc = bacc.Bacc(None, target_bir_lowering=False)

    x_d = nc.declare_dram_parameter("x", [N, H], F32, isOutput=False)
    w_d = {
        "q": nc.declare_dram_parameter("wq", [H, H], F32, isOutput=False),
        "k": nc.declare_dram_parameter("wk", [H, H], F32, isOutput=False),
        "v": nc.declare_dram_parameter("wv", [O, H], F32, isOutput=False),
    }
    out_d = nc.declare_dram_parameter("out", [NQ, O], F32, isOutput=True)

    with TileContext(nc) as tc:
        with (
            tc.tile_pool(name="const", bufs=1) as cpool,
            tc.tile_pool(name="big", bufs=1) as big,
            tc.tile_pool(name="stagea", bufs=1) as sta,
            tc.tile_pool(name="psum", bufs=2, space="PSUM") as psum,
            tc.tile_pool(name="apool", bufs=4) as apool,
            tc.tile_pool(name="epi", bufs=3) as epi,
        ):
            xTs = [big.tile([P, 512], BF16, name=f"xT{c}") for c in range(NC)]
            uTs = [big.tile([P, 512], BF16, name=f"uT{c}") for c in range(NQ // 512)]
            xcs = [big.tile([P, 512], BF16, name=f"xc{c}") for c in range(NC)]
            xf8 = [big.tile([P, 512], FP8E4, name=f"xf8{c}") for c in range(NC)]
            xst = [big.tile([P, 4, P], F32, name=f"xst{c}") for c in range(NC)]
            wvT = big.tile([P, P], BF16, name="wvT")
            m_sb = big.tile([P, P], BF16, name="m_sb")
            wf = sta.tile([P, 3 * P], F32, name="wf")

            # ---------- prep emitters ----------
            def chunk_load(c):
                rows = x_d[c * 512 : (c + 1) * 512, :]
                nc.sync.dma_start(
                    out=xst[c][:], in_=rows.rearrange("(t p) h -> p t h", p=P)
                )

            def chunk_cast(c, eng="vec"):
                src_ap = xst[c].rearrange("p t h -> p (t h)")
                if eng == "vec":
                    nc.vector.tensor_copy(xcs[c][:], src_ap)
                elif eng == "gp":
                    nc.gpsimd.tensor_copy(xcs[c][:], src_ap)
                else:
                    nc.scalar.activation(
                        xcs[c][:], src_ap, mybir.ActivationFunctionType.Copy
                    )

            def chunk_cast_f8(c, eng="vec"):
                src_ap = xst[c].rearrange("p t h -> p (t h)")
                if eng == "vec":
                    nc.vector.tensor_copy(xf8[c][:], src_ap)
                elif eng == "gp":
                    nc.gpsimd.tensor_copy(xf8[c][:], src_ap)
                else:
                    nc.scalar.activation(
                        xf8[c][:], src_ap, mybir.ActivationFunctionType.Copy
                    )

            def w_load():
                for wi, name in enumerate(("q", "k", "v")):
                    nc.scalar.dma_start(
                        out=wf[:, wi * P : (wi + 1) * P], in_=w_d[name][:]
                    )

            def emit_consts():
                ident_bf_ = cpool.tile([P, P], BF16, name="ident_bf")
                make_identity(nc, ident_bf_)
                ones8_ = cpool.tile([P, 2, 32], FP8E4, name="ones8")
                nc.gpsimd.memset(ones8_[:], 1.0)
                ones1_ = cpool.tile([1, 1], BF16, name="ones1")
                nc.gpsimd.memset(ones1_[:], 1.0)
                ebias_ = cpool.tile([P, 1], F32, name="ebias")
                nc.gpsimd.memset(ebias_[:], float(-4.0 * np.log(2.0)))
                return ident_bf_, ones8_, ones1_, ebias_

            def pe_transpose(c, copy_eng):
                px = psum.tile([P, 512], BF16, tag="aux", bufs=1, name=f"px{c}")
                for t4 in range(4):
                    nc.tensor.transpose(
                        px[:, t4 * P : (t4 + 1) * P],
                        xcs[c][:, t4 * P : (t4 + 1) * P],
                        ident_bf[:],
                    )
                if copy_eng == "act":
                    nc.scalar.activation(
                        xTs[c][:], px[:], mybir.ActivationFunctionType.Copy
                    )
                else:
                    nc.vector.tensor_copy(xTs[c][:], px[:])

            def xbar_transpose(c, q):
                # XBAR DMA transpose per 128x128 bf16 tile (~1.3us each);
                # all on the sync queue: they burn engine-track time on
                # whichever queue hosts them and ACT (scalar) is loaded
                eng = nc.sync
                for t4 in range(4):
                    eng.dma_start(
                        out=xTs[c][:, t4 * P : (t4 + 1) * P],
                        in_=xcs[c][:, t4 * P : (t4 + 1) * P],
                        transpose=True,
                    )

            wqk = big.tile([P, 2 * P], BF16, name="wqk")

            def w_casts():
                nc.scalar.activation(
                    wqk[:], wf[:, 0 : 2 * P],
                    mybir.ActivationFunctionType.Copy,
                )

            def m_bf16():
                pm = psum.tile([P, P], F32, tag="aux", bufs=1, name="pm")
                nc.tensor.matmul(
                    pm[:], wqk[:, 0:P], wqk[:, P : 2 * P], start=True, stop=True
                )
                nc.scalar.activation(
                    m_sb[:], pm[:], mybir.ActivationFunctionType.Copy
                )

            wst_holder = {}

            def wv_cast():
                wst = sta.tile([P, P], BF16, name="wst")
                nc.scalar.activation(
                    wst[:], wf[:, 2 * P : 3 * P],
                    mybir.ActivationFunctionType.Copy,
                )
                wst_holder["wst"] = wst

            def wv_transpose():
                pw = psum.tile([P, P], BF16, tag="aux", bufs=1, name="pw")
                nc.tensor.transpose(pw[:], wst_holder["wst"][:], ident_bf[:])
                nc.vector.tensor_copy(wvT[:], pw[:])

            def chunk_u(c, copy_eng="act"):
                # chunks 0/1 run in prep on the aux ring; chunks 2/3 are
                # emitted mid-block and borrow a slot of the "ps" ring
                tag = "aux" if c < 2 else "ps"
                bufs = 1 if c < 2 else 3
                pu = psum.tile([P, 512], F32, tag=tag, bufs=bufs, name=f"pu{c}")
                nc.tensor.matmul(pu[:], m_sb[:], xTs[c][:], start=True, stop=True)
                if copy_eng == "act":
                    nc.scalar.activation(
                        uTs[c][:], pu[:], mybir.ActivationFunctionType.Copy
                    )
                else:
                    nc.vector.tensor_copy(uTs[c][:], pu[:])

            # ---------- attention emitters ----------
            a_pairs = {}

            def emit_s_exp(qb, kt):
                # h0 exps always on ACT, h1 always on DVE: the ps-ring FIFO
                # chains exp(kt+1,h) -> S(kt+2,h) -> exp(kt+2,h), so each
                # half forms its own serial chain; pinning halves to engines
                # makes the two chains run concurrently instead of
                # serializing every exp through one combined chain.
                pr = kt // 2
                if kt % 2 == 0:
                    ap = apool.tile([P, 2, QBS], FP8E5, tag="a", name=f"a{qb}_{pr}")
                    a_pairs[(qb, pr)] = ap
                ap = a_pairs[(qb, pr)]
                for h in range(QBS // 512):
                    ps = psum.tile([P, 512], F32, tag="ps", bufs=3,
                                   name=f"s{qb}_{kt}_{h}")
                    nc.tensor.matmul(
                        ps[:], kslice(xTs, kt), uTs[qb * 2 + h][:],
                        start=True, stop=True,
                    )
                    if h == 1:
                        dst = ap.bitcast(UI8)[:, kt % 2, h * 512 : (h + 1) * 512]
                        nc.vector.tensor_scalar(
                            dst, ps[:], E5S0, E5S1,
                            mybir.AluOpType.mult, mybir.AluOpType.add,
                        )
                    else:
                        # true exp on ACT: same ACTIVATE-class cost as a
                        # scaled copy, no Schraudolph PWL error
                        dst = ap[:, kt % 2, h * 512 : (h + 1) * 512]
                        nc.scalar.activation(
                            dst, ps[:], mybir.ActivationFunctionType.Exp,
                            scale=float(SCALE), bias=ebias[:],
                        )

            def kslice(tiles, kt):
                return tiles[kt // 4][:, (kt % 4) * P : (kt % 4 + 1) * P]

            def xpair(p):
                c, sub = p // 2, p % 2
                return xf8[c][:, sub * 256 : (sub + 1) * 256].rearrange(
                    "p (a b) -> p a b", a=2
                )

            blk_state = {}

            def new_blk(qb):
                blk_state[qb] = {
                    "py": psum.tile([P, QBS], F32, tag="py", bufs=1,
                                    name=f"py{qb}"),
                    "pl0": psum.tile([32, 512], F32, tag="pl0", bufs=1,
                                     name=f"pl0_{qb}"),
                    "pl1": psum.tile([32, 512], F32, tag="pl1", bufs=1,
                                     name=f"pl1_{qb}"),
                }
                return blk_state[qb]

            def emit_pair(qb, p, b):
                ap = a_pairs.pop((qb, p))
                for h in range(2):
                    nc.tensor.matmul(
                        b["py"][:, h * 512 : (h + 1) * 512],
                        xpair(p),
                        ap[:, :, h * 512 : (h + 1) * 512],
                        start=(p == 0), stop=(p == NPAIR - 1),
                        perf_mode=mybir.MatmulPerfMode.DoubleRow,
                    )
                for h in range(2):
                    nc.tensor.matmul(
                        b[f"pl{h}"][:],
                        ones8[:],
                        ap[:, :, h * 512 : (h + 1) * 512],
                        start=(p == 0), stop=(p == NPAIR - 1),
                        perf_mode=mybir.MatmulPerfMode.DoubleRow,
                    )

            # ---------- epilogue ----------
            def epilogue_pieces(qb, b, last=False):
                st = {}

                def p_w():
                    w_sb = epi.tile([P, QBS], BF16, tag="w_sb", name=f"w{qb}")
                    nc.scalar.activation(
                        w_sb[:, 0:512], b["py"][:, 0:512],
                        mybir.ActivationFunctionType.Copy,
                    )
                    nc.vector.tensor_copy(
                        w_sb[:, 512:QBS], b["py"][:, 512:QBS]
                    )
                    st["w_sb"] = w_sb

                def p_lcopy(h):
                    # pl row 0 (all 32 rows identical) -> bf16 SBUF row
                    # (spread-matmul stationary must be bf16)
                    if h == 0:
                        st["lsb"] = epi.tile([1, QBS], BF16, tag="linv",
                                             name=f"li{qb}")
                        nc.scalar.activation(
                            st["lsb"][0:1, 0:512], b["pl0"][0:1, :],
                            mybir.ActivationFunctionType.Copy,
                        )
                    else:
                        nc.vector.tensor_copy(
                            st["lsb"][0:1, 512:QBS], b["pl1"][0:1, :]
                        )

                def p_spread(g):
                    # 4 tiny K=1 matmuls: [1,128] lhsT x [1,1] -> [128,1]
                    if g == 0:
                        st["plinv"] = psum.tile([P, 8], F32, tag="aux", bufs=1,
                                                name=f"plv{qb}")
                    for j in range(g * 4, g * 4 + 4):
                        nc.tensor.matmul(
                            st["plinv"][:, j : j + 1],
                            st["lsb"][0:1, j * P : (j + 1) * P],
                            ones1[:], start=True, stop=True,
                        )

                def p_lcol():
                    st["lcol"] = epi.tile([P, 8], F32, tag="lcol",
                                          name=f"lc{qb}")
                    nc.vector.reciprocal(st["lcol"][:], st["plinv"][:])

                def p_j(j):
                    g = j // 4
                    if j % 4 == 0:
                        st[f"yout{g}"] = epi.tile(
                            [P, 4, P], F32, tag="yout", name=f"yout{qb}_{g}"
                        )
                    psm_y = psum.tile([P, P], F32, tag="ps" if last else "aux",
                                       bufs=3 if last else 1)
                    nc.tensor.matmul(
                        psm_y[:], st["w_sb"][:, j * P : (j + 1) * P], wvT[:],
                        start=True, stop=True,
                    )
                    lc = st["lcol"][:, j : j + 1]
                    if j % 2 == 0:
                        nc.scalar.activation(
                            st[f"yout{g}"][:, j % 4, :], psm_y[:],
                            mybir.ActivationFunctionType.Copy, scale=lc,
                        )
                    else:
                        nc.vector.tensor_scalar_mul(
                            st[f"yout{g}"][:, j % 4, :], psm_y[:], lc
                        )
                    if not last:
                        if j % 4 == 3:
                            r0 = qb * QBS + g * 512
                            nc.sync.dma_start(
                                out=out_d[r0 : r0 + 512, :].rearrange(
                                    "(t p) h -> p t h", p=P
                                ),
                                in_=st[f"yout{g}"][:],
                            )
                    else:
                        if j % 2 == 1:
                            r0 = qb * QBS + (j - 1) * P
                            eng = nc.sync if (j // 2) % 2 == 0 else nc.scalar
                            eng.dma_start(
                                out=out_d[r0 : r0 + 256, :].rearrange(
                                    "(t p) h -> p t h", p=P
                                ),
                                in_=st[f"yout{g}"][:, (j - 1) % 4 : (j - 1) % 4 + 2, :],
                            )

                pieces = [p_w, lambda: p_lcopy(0), lambda: p_lcopy(1),
                          lambda: p_spread(0), lambda: p_spread(1), p_lcol]
                pieces += [lambda j=j: p_j(j) for j in range(8)]
                return pieces

            # ---------- emission schedule ----------
            chunk_load(0)
            chunk_load(1)
            w_load()
            ident_bf, ones8, ones1, ebias = emit_consts()
            for c in range(2, NC):
                chunk_load(c)
            w_casts()
            chunk_cast(0)
            chunk_cast(1)
            pe_transpose(0, "vec")
            pe_transpose(1, "vec")
            m_bf16()
            chunk_u(0, "act")
            chunk_u(1, "vec")
            chunk_cast_f8(0, "vec")

            prep_todo = [
                lambda: chunk_cast_f8(1, "vec"),
                lambda: chunk_cast(2, "gp"),
                lambda: xbar_transpose(2, 0),
                lambda: chunk_cast_f8(2, "gp"),
                lambda: chunk_cast(3, "gp"),
                lambda: xbar_transpose(3, 0),
                lambda: chunk_cast_f8(3, "gp"),
                wv_cast,
                wv_transpose,
                lambda: chunk_cast(4, "gp"),
                lambda: xbar_transpose(4, 0),
                lambda: chunk_cast_f8(4, "gp"),
                lambda: chunk_cast(5, "gp"),
                lambda: xbar_transpose(5, 0),
                lambda: chunk_cast_f8(5, "gp"),
                lambda: chunk_u(2, "act"),
                lambda: chunk_cast(6, "gp"),
                lambda: xbar_transpose(6, 0),
                lambda: chunk_cast_f8(6, "gp"),
                lambda: chunk_u(3, "vec"),
                lambda: chunk_cast(7, "gp"),
                lambda: xbar_transpose(7, 0),
                lambda: chunk_cast_f8(7, "gp"),
            ]

            blk = None
            todo = None
            for qb in range(QB):
                if qb == 0:
                    emit_s_exp(0, 0)
                    emit_s_exp(0, 1)
                    todo = prep_todo
                else:
                    todo = epilogue_pieces(qb - 1, blk)
                blk = new_blk(qb)

                for kt in range(KT):
                    if kt + 2 < KT:
                        emit_s_exp(qb, kt + 2)
                    elif qb + 1 < QB:
                        emit_s_exp(qb + 1, kt + 2 - KT)
                    if todo:
                        todo.pop(0)()
                    if kt % 2 == 1:
                        emit_pair(qb, (kt - 1) // 2, blk)
                while todo:
                    todo.pop(0)()

            for piece in epilogue_pieces(QB - 1, blk, last=True):
                piece()

    nc.compile()
    return nc


def _run(x, Wq, Wk, Wv, **spmd_kwargs):
    global _cached_nc
    if _cached_nc is None:
        _cached_nc = build_kernel()
    nc = _cached_nc

    x = np.asarray(x, dtype=np.float32)
    Wq = np.ascontiguousarray(np.asarray(Wq, dtype=np.float32))
    Wk = np.ascontiguousarray(np.asarray(Wk, dtype=np.float32))
    Wv = np.ascontiguousarray(np.asarray(Wv, dtype=np.float32))

    B = x.shape[0]
    in_maps = []
    for core in range(8):
        b, half = core // 2, core % 2
        xb = x[b]
        if half:
            xb = np.roll(xb, -NQ, axis=0)  # queries -> rows 0..NQ-1
        in_maps.append(
            {"x": np.ascontiguousarray(xb), "wq": Wq, "wk": Wk, "wv": Wv}
        )

    res = run_bass_kernel_spmd(nc, in_maps, core_ids=list(range(8)), **spmd_kwargs)

    y = np.empty((B, N, O), dtype=np.float32)
    for core in range(8):
        b, half = core // 2, core % 2
        y[b, half * NQ : (half + 1) * NQ] = res.results[core]["out"]
    return y, res


def kernel(x, Wq, Wk, Wv):
    y, _ = _run(x, Wq, Wk, Wv)
    return y


if __name__ == "__main__":
    rng = np.random.default_rng(0)
    x = rng.standard_normal((4, N, H), dtype=np.float32)
    Wq = rng.standard_normal((H, H), dtype=np.float32) / np.sqrt(H)
    Wk = rng.standard_normal((H, H), dtype=np.float32) / np.sqrt(H)
    Wv = rng.standard_normal((O, H), dtype=np.float32) / np.sqrt(H)
    y = kernel(x=x, Wq=Wq, Wk=Wk, Wv=Wv)
    print("kernel output", y.shape, y.dtype)
